# revision 14
# baseline (speedup 1.0000x reference)
"""Trainium2 Bass kernel for a 2-layer SRU (B=32, S=1024, D=256, H=512).

Strategy
--------
Data-parallel over batch: each of the 8 NeuronCores processes 4 batch
elements; weights are replicated.

Everything on-chip is kept in *channel-major* orientation [channel, (b, s)]:
  - the host pre-transposes each core's input shard to xT [D, BL*S], so the
    layer matmuls (stationary = W column block, moving = xT/h1T time columns)
    directly produce U^T in PSUM with channels on partitions,
  - the SRU recurrence c_t = f_t * c_{t-1} + (1 - f_t) * xt_t then maps 1:1
    onto the native DVE prefix-scan instruction (tensor_tensor_scan: one
    independent recurrence per partition, scanned along the free/time axis),
  - layer-1 output h1^T is already in the right orientation to be the moving
    operand of layer 2 (contraction over H on partitions), so no transposes
    are needed anywhere on-chip; the host un-transposes the final h2^T.

Elementwise tiles are bf16 (2x DVE perf mode + halved DRAIN); the scan's
internal state stays fp32 in hardware regardless of operand dtype, and the
scan output c is kept fp32 for the tanh. Matmul dtype is configurable
(bf16 with fast-weight-load, or float32r for full precision).
"""

import os
import numpy as np
import ml_dtypes

import concourse.bass as bass
import concourse.bacc as bacc
import concourse.mybir as mybir
import concourse.tile as tile

B, S, D, H = 32, 1024, 256, 512
NCORES = 8
BL = B // NCORES          # batch elements per core
NT = BL * S               # time columns per core (batch-concatenated)
KT1 = D // 128            # layer-1 contraction tiles
KT2 = H // 128            # layer-2 contraction tiles
JT = H // 128             # output-channel tiles per layer
NCH = S // 512            # 512-wide matmul column groups per batch

F32 = mybir.dt.float32
F32R = mybir.dt.float32r
BF16 = mybir.dt.bfloat16
AF = mybir.ActivationFunctionType
OP = mybir.AluOpType

# dtype knobs (accuracy / speed tradeoff)
DT_MM1 = BF16      # x / W1 matmul operands
DT_MM2 = BF16      # h1 / W2 matmul operands
DT_GATE = BF16     # f, bp (scan operands; scan state is fp32 internally)
DT_HIGH = BF16     # xt, r, tanh, highway ops
DT_OUT = BF16      # hT DRAM dtype (host converts back to fp32)

_NPDT = {F32: np.float32, F32R: np.float32, BF16: ml_dtypes.bfloat16}


def build():
    """Build the Bass program (identical for every core)."""
    nc = bacc.Bacc("TRN2", target_bir_lowering=False, debug=False,
                   enable_asserts=True)

    xT_d = nc.dram_tensor("xT", [D, NT], DT_MM1, kind="ExternalInput").ap()
    w1_d = nc.dram_tensor("W1", [D, 3 * H], DT_MM1, kind="ExternalInput").ap()
    w2_d = nc.dram_tensor("W2", [H, 3 * H], DT_MM2, kind="ExternalInput").ap()
    bf1_d = nc.dram_tensor("bf1", [H], F32, kind="ExternalInput").ap()
    br1_d = nc.dram_tensor("br1", [H], F32, kind="ExternalInput").ap()
    bf2_d = nc.dram_tensor("bf2", [H], F32, kind="ExternalInput").ap()
    br2_d = nc.dram_tensor("br2", [H], F32, kind="ExternalInput").ap()
    bf1n_d = nc.dram_tensor("bf1n", [H], F32, kind="ExternalInput").ap()
    bf2n_d = nc.dram_tensor("bf2n", [H], F32, kind="ExternalInput").ap()
    hT_d = nc.dram_tensor("hT", [H, NT], DT_OUT, kind="ExternalOutput").ap()

    with tile.TileContext(nc) as tc:
        with tc.tile_pool(name="persist", bufs=1) as persist, \
             tc.tile_pool(name="f", bufs=5) as fpool, \
             tc.tile_pool(name="xt", bufs=6) as xtpool, \
             tc.tile_pool(name="bp", bufs=5) as bppool, \
             tc.tile_pool(name="g", bufs=5) as gpool, \
             tc.tile_pool(name="c", bufs=5) as cpool, \
             tc.tile_pool(name="th", bufs=5) as thpool, \
             tc.tile_pool(name="d", bufs=5) as dpool, \
             tc.tile_pool(name="e", bufs=5) as epool, \
             tc.tile_pool(name="r", bufs=5) as rpool, \
             tc.tile_pool(name="h2", bufs=5) as h2pool, \
             tc.tile_pool(name="ps_f", bufs=3, space="PSUM") as psf_pool, \
             tc.tile_pool(name="ps_xr", bufs=5, space="PSUM") as psxr_pool:

            def load_bias(dram_ap, name):
                view = dram_ap.rearrange("(j p one) -> j p one", p=128, one=1)
                out = []
                for j in range(JT):
                    t = persist.tile([128, 1], F32, tag=f"{name}_{j}",
                                     name=f"{name}_{j}")
                    nc.sync.dma_start(t[:], view[j])
                    out.append(t)
                return out

            # layer-1 inputs first (layer-2 weights deferred below)
            w1_t = []
            for k in range(KT1):
                w = persist.tile([128, 3 * H], DT_MM1, tag=f"w1_{k}",
                                 name=f"w1_{k}")
                nc.sync.dma_start(w[:], w1_d[k * 128:(k + 1) * 128, :])
                w1_t.append(w)
            bf1_t = load_bias(bf1_d, "bf1")
            br1_t = load_bias(br1_d, "br1")
            bf1n_t = load_bias(bf1n_d, "bf1n")

            xT_t = [[None] * BL for _ in range(KT1)]
            for b in range(BL):
                for k in range(KT1):
                    t = persist.tile([128, S], DT_MM1, tag=f"xT_{k}_{b}",
                                     name=f"xT_{k}_{b}")
                    nc.sync.dma_start(
                        t[:], xT_d[k * 128:(k + 1) * 128, b * S:(b + 1) * S])
                    xT_t[k][b] = t

            h1_t = [[None] * BL for _ in range(JT)]
            for j in range(JT):
                for b in range(BL):
                    h1_t[j][b] = persist.tile([128, S], DT_MM2,
                                              tag=f"h1_{j}_{b}",
                                              name=f"h1_{j}_{b}")

            def emit_front(j, b, rhs, w_t, bf_t, bfn_t, br_t):
                """Matmuls + gates + scan for one (channel-tile, batch) unit."""
                kt = len(w_t)
                f_t = fpool.tile([128, S], DT_GATE)
                xt_t = xtpool.tile([128, S], DT_HIGH)
                r_t = rpool.tile([128, S], DT_HIGH)
                # per-512-chunk psum tiles (1 bank each) for finer rotation
                g_t = gpool.tile([128, S], DT_GATE)
                for ch, m, dst, bias in (
                        ("f", 4 + j, f_t, bf_t[j]),
                        ("x", j, xt_t, None),
                        ("r", 8 + j, r_t, br_t[j])):
                    pool = psf_pool if ch == "f" else psxr_pool
                    for n in range(NCH):
                        ps = pool.tile([128, 512], F32,
                                       tag=pool.name, name=f"ps_{ch}{n}")
                        for k in range(kt):
                            nc.tensor.matmul(
                                ps[:],
                                w_t[k][:, m * 128:(m + 1) * 128],
                                rhs[k][b][:, n * 512:(n + 1) * 512],
                                start=(k == 0), stop=(k == kt - 1))
                        dslice = dst[:, n * 512:(n + 1) * 512]
                        if bias is None:
                            nc.scalar.copy(dslice, ps[:])
                        else:
                            nc.scalar.activation(dslice, ps[:], AF.Sigmoid,
                                                 bias=bias[:])
                        if ch == "f":
                            # g = 1 - f = sigmoid(-(U_f + bf)), same psum chunk
                            nc.scalar.activation(
                                g_t[:, n * 512:(n + 1) * 512], ps[:],
                                AF.Sigmoid, bias=bfn_t[j][:], scale=-1.0)

                # bp = (1 - f) * xt ; scan: c = f*c + bp
                bp_t = bppool.tile([128, S], DT_GATE)
                nc.vector.tensor_mul(bp_t[:], g_t[:], xt_t[:])
                c_t = cpool.tile([128, S], F32)
                nc.vector.tensor_tensor_scan(
                    c_t[:], f_t[:], bp_t[:], 0.0,
                    op0=OP.mult, op1=OP.add)
                return (c_t, xt_t, r_t)

            def emit_back(state, out_tile, dma_dst):
                """Highway + output for a unit (emitted one unit later)."""
                c_t, xt_t, r_t = state
                # h = r*tanh(c) + (1-r)*xt = (tanh(c) - xt)*r + xt
                th_t = thpool.tile([128, S], DT_HIGH)
                nc.scalar.activation(th_t[:], c_t[:], AF.Tanh)
                d_t = dpool.tile([128, S], DT_HIGH)
                nc.vector.tensor_sub(d_t[:], th_t[:], xt_t[:])
                e_t = epool.tile([128, S], DT_HIGH)
                nc.gpsimd.tensor_mul(e_t[:], r_t[:], d_t[:])
                h_t = out_tile if out_tile is not None else \
                    h2pool.tile([128, S], DT_OUT)
                nc.gpsimd.tensor_add(h_t[:], e_t[:], xt_t[:])
                if dma_dst is not None:
                    nc.sync.dma_start(dma_dst, h_t[:])

            # software-pipelined unit loop: back(u-1) emitted between
            # front(u) and front(u+1) so no engine stream head-of-line blocks
            pending = None   # (state, out_tile, dma_dst)

            # layer 1: rhs = xT, output -> h1 tiles
            for b in range(BL):
                for j in range(JT):
                    st = emit_front(j, b, xT_t, w1_t, bf1_t, bf1n_t, br1_t)
                    if pending is not None:
                        emit_back(*pending)
                    pending = (st, h1_t[j][b], None)

            # layer-2 weights (emitted late so their DMA doesn't delay x/W1)
            w2_t = []
            for k in range(KT2):
                w = persist.tile([128, 3 * H], DT_MM2, tag=f"w2_{k}",
                                 name=f"w2_{k}")
                nc.sync.dma_start(w[:], w2_d[k * 128:(k + 1) * 128, :])
                w2_t.append(w)
            bf2_t = load_bias(bf2_d, "bf2")
            br2_t = load_bias(br2_d, "br2")
            bf2n_t = load_bias(bf2n_d, "bf2n")

            # layer 2: rhs = h1, output -> DRAM hT
            for b in range(BL):
                for j in range(JT):
                    st = emit_front(j, b, h1_t, w2_t, bf2_t, bf2n_t, br2_t)
                    if pending is not None:
                        emit_back(*pending)
                    pending = (st, None,
                               hT_d[j * 128:(j + 1) * 128, b * S:(b + 1) * S])
            emit_back(*pending)
    nc.compile()
    return nc


_NC_CACHE = {}


def kernel(x, W1, bf1, br1, W2, bf2, br2):
    from concourse.bass_utils import run_bass_kernel_spmd

    x = np.asarray(x, np.float32)
    W1 = np.asarray(W1, np.float32)
    W2 = np.asarray(W2, np.float32)
    bf1 = np.asarray(bf1, np.float32)
    br1 = np.asarray(br1, np.float32)
    bf2 = np.asarray(bf2, np.float32)
    br2 = np.asarray(br2, np.float32)

    if "nc" not in _NC_CACHE:
        _NC_CACHE["nc"] = build()
    nc = _NC_CACHE["nc"]

    W1c = W1.astype(_NPDT[DT_MM1])
    W2c = W2.astype(_NPDT[DT_MM2])
    in_maps = []
    for c in range(NCORES):
        xs = x[c * BL:(c + 1) * BL]                      # [BL, S, D]
        xT = np.ascontiguousarray(
            xs.transpose(2, 0, 1).reshape(D, NT)).astype(_NPDT[DT_MM1])
        in_maps.append({"xT": xT, "W1": W1c, "W2": W2c,
                        "bf1": bf1, "br1": br1, "bf2": bf2, "br2": br2,
                        "bf1n": -bf1, "bf2n": -bf2})

    trace = bool(int(os.environ.get("BASS_KERNEL_TRACE", "0")))
    res = run_bass_kernel_spmd(nc, in_maps, list(range(NCORES)), trace=trace)
    kernel.last_result = res

    hidden = np.empty((B, S, H), np.float32)
    for c in range(NCORES):
        hT = np.asarray(res.results[c]["hT"], dtype=np.float32)  # [H, NT]
        hidden[c * BL:(c + 1) * BL] = (
            hT.reshape(H, BL, S).transpose(1, 2, 0))
    out = np.ascontiguousarray(hidden[:, -1, :])
    return out, hidden


# revision 15
# speedup vs baseline: 1.2456x; 1.2456x over previous
"""Trainium2 Bass kernel for a 2-layer SRU (B=32, S=1024, D=256, H=512).

Strategy
--------
Data-parallel over batch: each of the 8 NeuronCores processes 4 batch
elements; weights are replicated.

Everything on-chip is kept in *channel-major* orientation [channel, (b, s)]:
  - the host pre-transposes each core's input shard to xT [D, BL*S], so the
    layer matmuls (stationary = W column block, moving = xT/h1T time columns)
    directly produce U^T in PSUM with channels on partitions,
  - the SRU recurrence c_t = f_t * c_{t-1} + (1 - f_t) * xt_t then maps 1:1
    onto the native DVE prefix-scan instruction (tensor_tensor_scan: one
    independent recurrence per partition, scanned along the free/time axis),
  - layer-1 output h1^T is already in the right orientation to be the moving
    operand of layer 2 (contraction over H on partitions), so no transposes
    are needed anywhere on-chip; the host un-transposes the final h2^T.

Elementwise tiles are bf16 (2x DVE perf mode + halved DRAIN); the scan's
internal state stays fp32 in hardware regardless of operand dtype, and the
scan output c is kept fp32 for the tanh. Matmul dtype is configurable
(bf16 with fast-weight-load, or float32r for full precision).
"""

import os
import numpy as np
import ml_dtypes

import concourse.bass as bass
import concourse.bacc as bacc
import concourse.mybir as mybir
import concourse.tile as tile

B, S, D, H = 32, 1024, 256, 512
NCORES = 8
BL = B // NCORES          # batch elements per core
NT = BL * S               # time columns per core (batch-concatenated)
KT1 = D // 128            # layer-1 contraction tiles
KT2 = H // 128            # layer-2 contraction tiles
JT = H // 128             # output-channel tiles per layer
NCH = S // 512            # 512-wide matmul column groups per batch

F32 = mybir.dt.float32
F32R = mybir.dt.float32r
BF16 = mybir.dt.bfloat16
AF = mybir.ActivationFunctionType
OP = mybir.AluOpType

# dtype knobs (accuracy / speed tradeoff)
DT_MM1 = BF16      # x / W1 matmul operands
DT_MM2 = BF16      # h1 / W2 matmul operands
DT_GATE = BF16     # f, bp (scan operands; scan state is fp32 internally)
DT_HIGH = BF16     # xt, r, tanh, highway ops
DT_OUT = BF16      # hT DRAM dtype (host converts back to fp32)

_NPDT = {F32: np.float32, F32R: np.float32, BF16: ml_dtypes.bfloat16}


def build():
    """Build the Bass program (identical for every core)."""
    nc = bacc.Bacc("TRN2", target_bir_lowering=False, debug=False,
                   enable_asserts=True)

    xT_d = nc.dram_tensor("xT", [D, NT], DT_MM1, kind="ExternalInput").ap()
    w1_d = nc.dram_tensor("W1", [D, 3 * H], DT_MM1, kind="ExternalInput").ap()
    w2_d = nc.dram_tensor("W2", [H, 3 * H], DT_MM2, kind="ExternalInput").ap()
    bf1_d = nc.dram_tensor("bf1", [H], F32, kind="ExternalInput").ap()
    br1_d = nc.dram_tensor("br1", [H], F32, kind="ExternalInput").ap()
    bf2_d = nc.dram_tensor("bf2", [H], F32, kind="ExternalInput").ap()
    br2_d = nc.dram_tensor("br2", [H], F32, kind="ExternalInput").ap()
    bf1n_d = nc.dram_tensor("bf1n", [H], F32, kind="ExternalInput").ap()
    bf2n_d = nc.dram_tensor("bf2n", [H], F32, kind="ExternalInput").ap()
    hT_d = nc.dram_tensor("hT", [H, NT], DT_OUT, kind="ExternalOutput").ap()

    with tile.TileContext(nc) as tc:
        with tc.tile_pool(name="persist", bufs=1) as persist, \
             tc.tile_pool(name="f", bufs=5) as fpool, \
             tc.tile_pool(name="xt", bufs=6) as xtpool, \
             tc.tile_pool(name="bp", bufs=5) as bppool, \
             tc.tile_pool(name="g", bufs=5) as gpool, \
             tc.tile_pool(name="c", bufs=5) as cpool, \
             tc.tile_pool(name="th", bufs=5) as thpool, \
             tc.tile_pool(name="d", bufs=5) as dpool, \
             tc.tile_pool(name="e", bufs=5) as epool, \
             tc.tile_pool(name="r", bufs=5) as rpool, \
             tc.tile_pool(name="h2", bufs=5) as h2pool, \
             tc.tile_pool(name="ps_f", bufs=3, space="PSUM") as psf_pool, \
             tc.tile_pool(name="ps_xr", bufs=5, space="PSUM") as psxr_pool:

            def load_bias(dram_ap, name):
                view = dram_ap.rearrange("(j p one) -> j p one", p=128, one=1)
                out = []
                for j in range(JT):
                    t = persist.tile([128, 1], F32, tag=f"{name}_{j}",
                                     name=f"{name}_{j}")
                    nc.sync.dma_start(t[:], view[j])
                    out.append(t)
                return out

            # layer-1 inputs first (layer-2 weights deferred below)
            w1_t = []
            for k in range(KT1):
                w = persist.tile([128, 3 * H], DT_MM1, tag=f"w1_{k}",
                                 name=f"w1_{k}")
                nc.sync.dma_start(w[:], w1_d[k * 128:(k + 1) * 128, :])
                w1_t.append(w)
            bf1_t = load_bias(bf1_d, "bf1")
            br1_t = load_bias(br1_d, "br1")
            bf1n_t = load_bias(bf1n_d, "bf1n")

            xT_t = [[None] * BL for _ in range(KT1)]
            for b in range(BL):
                for k in range(KT1):
                    t = persist.tile([128, S], DT_MM1, tag=f"xT_{k}_{b}",
                                     name=f"xT_{k}_{b}")
                    nc.sync.dma_start(
                        t[:], xT_d[k * 128:(k + 1) * 128, b * S:(b + 1) * S])
                    xT_t[k][b] = t

            h1_t = [[None] * BL for _ in range(JT)]
            for j in range(JT):
                for b in range(BL):
                    h1_t[j][b] = persist.tile([128, S], DT_MM2,
                                              tag=f"h1_{j}_{b}",
                                              name=f"h1_{j}_{b}")

            def emit_front(j, b, rhs, w_t, bf_t, bfn_t, br_t):
                """Matmuls + gates + scan for one (channel-tile, batch) unit."""
                kt = len(w_t)
                f_t = fpool.tile([128, S], DT_GATE)
                xt_t = xtpool.tile([128, S], DT_HIGH)
                r_t = rpool.tile([128, S], DT_HIGH)
                # per-512-chunk psum tiles (1 bank each) for finer rotation
                g_t = gpool.tile([128, S], DT_GATE)
                for ch, m, dst, bias in (
                        ("f", 4 + j, f_t, bf_t[j]),
                        ("x", j, xt_t, None),
                        ("r", 8 + j, r_t, br_t[j])):
                    pool = psf_pool if ch == "f" else psxr_pool
                    for n in range(NCH):
                        ps = pool.tile([128, 512], F32,
                                       tag=pool.name, name=f"ps_{ch}{n}")
                        for k in range(kt):
                            nc.tensor.matmul(
                                ps[:],
                                w_t[k][:, m * 128:(m + 1) * 128],
                                rhs[k][b][:, n * 512:(n + 1) * 512],
                                start=(k == 0), stop=(k == kt - 1))
                        dslice = dst[:, n * 512:(n + 1) * 512]
                        if bias is None:
                            nc.scalar.copy(dslice, ps[:])
                        else:
                            nc.scalar.activation(dslice, ps[:], AF.Sigmoid,
                                                 bias=bias[:])
                        if ch == "f":
                            # g = 1 - f = sigmoid(-(U_f + bf)), same psum chunk
                            nc.scalar.activation(
                                g_t[:, n * 512:(n + 1) * 512], ps[:],
                                AF.Sigmoid, bias=bfn_t[j][:], scale=-1.0)

                # bp = (1 - f) * xt ; scan: c = f*c + bp
                bp_t = bppool.tile([128, S], DT_GATE)
                nc.vector.tensor_mul(bp_t[:], g_t[:], xt_t[:])
                c_t = cpool.tile([128, S], F32)
                nc.vector.tensor_tensor_scan(
                    c_t[:], f_t[:], bp_t[:], 0.0,
                    op0=OP.mult, op1=OP.add)
                return (c_t, xt_t, r_t)

            def emit_back(state, out_tile, dma_dst):
                """Highway + output for a unit (emitted one unit later)."""
                c_t, xt_t, r_t = state
                # h = r*tanh(c) + (1-r)*xt = (tanh(c) - xt)*r + xt
                th_t = thpool.tile([128, S], DT_HIGH)
                nc.scalar.activation(th_t[:], c_t[:], AF.Tanh)
                d_t = dpool.tile([128, S], DT_HIGH)
                nc.vector.tensor_sub(d_t[:], th_t[:], xt_t[:])
                e_t = epool.tile([128, S], DT_HIGH)
                nc.vector.tensor_mul(e_t[:], r_t[:], d_t[:])
                h_t = out_tile if out_tile is not None else \
                    h2pool.tile([128, S], DT_OUT)
                nc.vector.tensor_add(h_t[:], e_t[:], xt_t[:])
                if dma_dst is not None:
                    nc.sync.dma_start(dma_dst, h_t[:])

            # software-pipelined unit loop: back(u-1) emitted between
            # front(u) and front(u+1) so no engine stream head-of-line blocks
            pending = None   # (state, out_tile, dma_dst)

            # layer 1: rhs = xT, output -> h1 tiles
            for b in range(BL):
                for j in range(JT):
                    st = emit_front(j, b, xT_t, w1_t, bf1_t, bf1n_t, br1_t)
                    if pending is not None:
                        emit_back(*pending)
                    pending = (st, h1_t[j][b], None)

            # layer-2 weights (emitted late so their DMA doesn't delay x/W1)
            w2_t = []
            for k in range(KT2):
                w = persist.tile([128, 3 * H], DT_MM2, tag=f"w2_{k}",
                                 name=f"w2_{k}")
                nc.sync.dma_start(w[:], w2_d[k * 128:(k + 1) * 128, :])
                w2_t.append(w)
            bf2_t = load_bias(bf2_d, "bf2")
            br2_t = load_bias(br2_d, "br2")
            bf2n_t = load_bias(bf2n_d, "bf2n")

            # layer 2: rhs = h1, output -> DRAM hT
            for b in range(BL):
                for j in range(JT):
                    st = emit_front(j, b, h1_t, w2_t, bf2_t, bf2n_t, br2_t)
                    if pending is not None:
                        emit_back(*pending)
                    pending = (st, None,
                               hT_d[j * 128:(j + 1) * 128, b * S:(b + 1) * S])
            emit_back(*pending)
    nc.compile()
    return nc


_NC_CACHE = {}


def kernel(x, W1, bf1, br1, W2, bf2, br2):
    from concourse.bass_utils import run_bass_kernel_spmd

    x = np.asarray(x, np.float32)
    W1 = np.asarray(W1, np.float32)
    W2 = np.asarray(W2, np.float32)
    bf1 = np.asarray(bf1, np.float32)
    br1 = np.asarray(br1, np.float32)
    bf2 = np.asarray(bf2, np.float32)
    br2 = np.asarray(br2, np.float32)

    if "nc" not in _NC_CACHE:
        _NC_CACHE["nc"] = build()
    nc = _NC_CACHE["nc"]

    W1c = W1.astype(_NPDT[DT_MM1])
    W2c = W2.astype(_NPDT[DT_MM2])
    in_maps = []
    for c in range(NCORES):
        xs = x[c * BL:(c + 1) * BL]                      # [BL, S, D]
        xT = np.ascontiguousarray(
            xs.transpose(2, 0, 1).reshape(D, NT)).astype(_NPDT[DT_MM1])
        in_maps.append({"xT": xT, "W1": W1c, "W2": W2c,
                        "bf1": bf1, "br1": br1, "bf2": bf2, "br2": br2,
                        "bf1n": -bf1, "bf2n": -bf2})

    trace = bool(int(os.environ.get("BASS_KERNEL_TRACE", "0")))
    res = run_bass_kernel_spmd(nc, in_maps, list(range(NCORES)), trace=trace)
    kernel.last_result = res

    hidden = np.empty((B, S, H), np.float32)
    for c in range(NCORES):
        hT = np.asarray(res.results[c]["hT"], dtype=np.float32)  # [H, NT]
        hidden[c * BL:(c + 1) * BL] = (
            hT.reshape(H, BL, S).transpose(1, 2, 0))
    out = np.ascontiguousarray(hidden[:, -1, :])
    return out, hidden
